# revision 1
# baseline (speedup 1.0000x reference)
"""Trainium2 Bass kernel for nn_AggFeatureModel (segment_reduce).

Computes, per batch row b (B=4096, T=2048):
  - seq_len, sum/mean/std of amount over the full T axis
  - per-category (mcc: C=100, tr_type: C=50) count/mean/std of amount
  - distinct-category counts
Output: [B, 456] = [sl, s, mean, std, mcc_cnt(100), mcc_mean(100),
  mcc_std(100), tr_cnt(50), tr_mean(50), tr_std(50), dist_mcc, dist_tr]

Sharding: pure data parallel, B split across 8 NeuronCores (512 rows each).

Per-category work is split across three engines (HW-measured per-op costs on
[128, 2048] bf16 tiles):
  - DVE  tensor_scalar(is_eq,+0)+accum  -> mask (out) + count (accum), and
         the s-sum over masked_a (tensor_scalar mult+accum)
  - Pool (gpsimd) tensor_tensor mult    -> masked_a = mask * a for most
         categories (~3.2us/op; Pool only supports TT arithmetic on HW)
  - DVE  tensor_tensor mult             -> masked_a for the rest
  - Act  Square(masked_a)+accum         -> ss-sums
Consumers of Pool-produced masked_a are deferred a few iterations so the
in-order DVE/Act instruction streams don't stall on the slower Pool op.
"""

import os
import sys

sys.path.insert(0, "/opt/trn_rl_repo")

from contextlib import ExitStack

import numpy as np

import concourse.bass as bass
import concourse.tile as tile
from concourse import bacc, mybir
from concourse.bass_utils import run_bass_kernel_spmd

B, T = 4096, 2048
NCORES = 8
RPC = B // NCORES  # rows per core
C_MCC, C_TR = 100, 50
EPS = 1e-9
OUT_COLS = 456
PT = 128  # partition tile (rows per SBUF tile)
NT = RPC // PT  # row tiles per core

F32 = mybir.dt.float32
BF16 = mybir.dt.bfloat16
I32 = mybir.dt.int32
AX = mybir.AxisListType.X
OP = mybir.AluOpType
AF = mybir.ActivationFunctionType

# engine split (per 128-row tile, 146 scanned categories; the last category
# of each group is derived from row totals: sum_c s_c == sum_t a, etc.)
# Pool (gpsimd) only supports tensor_tensor arithmetic on HW, so its role is
# computing masked_a = mask * a for a subset of categories.
N_MA_POOL = int(os.environ.get("K_MA_POOL", "96"))  # masked_a products on Pool
N_SS_DVE = int(os.environ.get("K_SS_DVE", "10"))  # ss-sums on DVE via ma*ma
N_MA = int(os.environ.get("K_N_MA", "12"))   # masked_a rotation buffers
N_MSK = int(os.environ.get("K_N_MSK", "8"))  # mask rotation buffers
DEFER = int(os.environ.get("K_DEFER", "3"))  # defer of Pool-ma consumers
DEFER_DVE = int(os.environ.get("K_DEFER_DVE", "0"))  # defer of DVE-ma consumers
INTERLEAVE = bool(int(os.environ.get("K_INTERLEAVE", "0")))  # mix mcc/tr cats


def _bresenham(n, k):
    """k of n indices, evenly spread."""
    return {i for i in range(n) if (i + 1) * k // n > i * k // n}


def _cat_stats_postproc(nc, pool, cnt, s, ss, out_tile, col0, C):
    """Given per-category cnt/s/ss [128, C] f32, write cnt/mean/std into
    out_tile columns [col0:col0+3C] and return distinct count [128,1]."""
    tmp = pool.tile([PT, C], F32, tag=f"pp_tmp_{C}")
    rec = pool.tile([PT, C], F32, tag=f"pp_rec_{C}")
    # cnt goes out directly
    nc.vector.tensor_copy(out_tile[:, col0 : col0 + C], cnt[:])
    # rec = 1/(cnt + EPS)
    nc.vector.tensor_scalar(tmp[:], cnt[:], EPS, None, OP.add)
    nc.vector.reciprocal(rec[:], tmp[:])
    # mean = s * rec
    mean = out_tile[:, col0 + C : col0 + 2 * C]
    nc.vector.tensor_tensor(mean, s[:], rec[:], OP.mult)
    # var_num = clip(ss - s*mean, 0)
    nc.vector.tensor_tensor(tmp[:], s[:], mean, OP.mult)
    nc.vector.tensor_tensor(tmp[:], ss[:], tmp[:], OP.subtract)
    nc.vector.tensor_scalar(tmp[:], tmp[:], 0.0, None, OP.max)
    # denom = clip(cnt-1, 0) + EPS ; rec = 1/denom
    nc.vector.tensor_scalar(rec[:], cnt[:], 1.0, 0.0, OP.subtract, OP.max)
    nc.vector.tensor_scalar(rec[:], rec[:], EPS, None, OP.add)
    nc.vector.reciprocal(rec[:], rec[:])
    nc.vector.tensor_tensor(tmp[:], tmp[:], rec[:], OP.mult)
    nc.scalar.sqrt(out_tile[:, col0 + 2 * C : col0 + 3 * C], tmp[:])
    # distinct = sum(cnt > 0)
    dist = pool.tile([PT, 1], F32, tag=f"pp_dist_{C}")
    nc.vector.tensor_scalar(tmp[:], cnt[:], 0.0, None, OP.is_gt)
    nc.vector.reduce_sum(dist[:], tmp[:], axis=AX)
    return dist


def _build_body(
    ctx,
    tc,
    reps=1,
    ma_pool=None,
    ss_dve=None,
    defer=None,
    defer_dve=None,
    n_ma=None,
    n_msk=None,
    interleave=None,
):
    ma_pool = N_MA_POOL if ma_pool is None else ma_pool
    ss_dve = N_SS_DVE if ss_dve is None else ss_dve
    defer = DEFER if defer is None else defer
    defer_dve = DEFER_DVE if defer_dve is None else defer_dve
    n_ma = N_MA if n_ma is None else n_ma
    n_msk = N_MSK if n_msk is None else n_msk
    nc = tc.nc
    amount_d = nc.dram_tensor("amount", [RPC, T], F32, kind="ExternalInput")
    mcc_d = nc.dram_tensor("mcc", [RPC, T], I32, kind="ExternalInput")
    tr_d = nc.dram_tensor("tr_type", [RPC, T], I32, kind="ExternalInput")
    seq_d = nc.dram_tensor("seq_lens", [RPC, 1], I32, kind="ExternalInput")
    out_d = nc.dram_tensor("out", [RPC, OUT_COLS], F32, kind="ExternalOutput")

    io_pool = ctx.enter_context(tc.tile_pool(name="io", bufs=2))
    work = ctx.enter_context(tc.tile_pool(name="work", bufs=2))
    acc_pool = ctx.enter_context(tc.tile_pool(name="acc", bufs=2))
    scr_pool = ctx.enter_context(tc.tile_pool(name="scr", bufs=1))

    # category list and engine assignment (same for every tile); the last
    # category of each group (99 / 49) is derived from row totals instead
    if interleave is None:
        interleave = INTERLEAVE
    cats_m = [("m", c) for c in range(1, C_MCC - 1)]
    cats_t = [("t", c) for c in range(1, C_TR - 1)]
    if interleave:
        # round-robin mcc/tr so consecutive ops read different cat tensors
        cats = []
        im = it_ = 0
        for k in range(len(cats_m) + len(cats_t)):
            if it_ < len(cats_t) and (k % 3 == 2 or im >= len(cats_m)):
                cats.append(cats_t[it_])
                it_ += 1
            else:
                cats.append(cats_m[im])
                im += 1
    else:
        cats = cats_m + cats_t
    ncat = len(cats)
    ma_pool_set = _bresenham(ncat, ma_pool)
    ss_dve_set = _bresenham(ncat, ss_dve)

    # persistent junk-output scratch (WAW within one engine is in-order)
    scr_cnt = scr_pool.tile([PT, T], BF16, tag="scr_cnt")
    scr_act = scr_pool.tile([PT, T], BF16, tag="scr_act")
    scr_ss = scr_pool.tile([PT, T], BF16, tag="scr_ss")
    ma = [
        scr_pool.tile([PT, T], BF16, tag=f"ma{i}", name=f"ma{i}")
        for i in range(n_ma)
    ]
    msk = [
        scr_pool.tile([PT, T], BF16, tag=f"msk{i}", name=f"msk{i}")
        for i in range(n_msk)
    ]

    gidx = 0  # global masked_a rotation counter
    for it in range(NT * reps):
        it = it % NT
        r0 = it * PT
        rows = slice(r0, r0 + PT)

        a = io_pool.tile([PT, T], F32, tag="a")
        nc.sync.dma_start(a[:], amount_d[rows, :])
        mcc_i = io_pool.tile([PT, T], I32, tag="mcc_i")
        nc.sync.dma_start(mcc_i[:], mcc_d[rows, :])
        tr_i = io_pool.tile([PT, T], I32, tag="tr_i")
        nc.sync.dma_start(tr_i[:], tr_d[rows, :])
        seq_i = io_pool.tile([PT, 1], I32, tag="seq_i")
        nc.sync.dma_start(seq_i[:], seq_d[rows, :])

        # conversions (DVE): int32 -> bf16 directly, f32 amount -> bf16
        mcc_bf = work.tile([PT, T], BF16, tag="mcc_bf")
        nc.vector.tensor_copy(mcc_bf[:], mcc_i[:])
        tr_bf = work.tile([PT, T], BF16, tag="tr_bf")
        nc.vector.tensor_copy(tr_bf[:], tr_i[:])
        a_bf = work.tile([PT, T], BF16, tag="a_bf")
        nc.vector.tensor_copy(a_bf[:], a[:])
        seq_f = work.tile([PT, 1], F32, tag="seq_f")
        nc.vector.tensor_copy(seq_f[:], seq_i[:])

        # row sum of squares on Act; row sum on DVE (4x TS)
        ss_row = work.tile([PT, 1], F32, tag="ss_row")
        nc.scalar.activation(scr_act[:], a[:], AF.Square, accum_out=ss_row[:])
        s_row = work.tile([PT, 1], F32, tag="s_row")
        nc.vector.tensor_scalar(
            scr_cnt[:], a_bf[:], 1.0, 0.0, OP.mult, OP.add, accum_out=s_row[:]
        )

        # Per-category accumulators
        cnt_m = acc_pool.tile([PT, C_MCC], F32, tag="cnt_m")
        s_m = acc_pool.tile([PT, C_MCC], F32, tag="s_m")
        ss_m = acc_pool.tile([PT, C_MCC], F32, tag="ss_m")
        cnt_t = acc_pool.tile([PT, C_TR], F32, tag="cnt_t")
        s_t = acc_pool.tile([PT, C_TR], F32, tag="s_t")
        ss_t = acc_pool.tile([PT, C_TR], F32, tag="ss_t")
        for t_ in (cnt_m, s_m, ss_m, cnt_t, s_t, ss_t):
            nc.vector.memset(t_[:, 0:1], 0.0)

        # Per category: DVE tensor_scalar builds the mask (out) + count
        # (accum); masked_a = mask*a on DVE or Pool; s = DVE TS-accum over
        # masked_a; ss = Act Square-accum over masked_a. Consumers of
        # Pool-produced masked_a are deferred DEFER iterations so the
        # in-order DVE/Act streams don't stall on the slower Pool op.
        pending = {}  # emit_step -> list of (mab, s_ap, ss_ap, ss_on_dve)

        def emit_consumers(mab, s_ap, ss_ap, ss_on_dve):
            nc.vector.tensor_scalar(
                scr_cnt[:], mab[:], 1.0, 0.0, OP.mult, OP.add, accum_out=s_ap
            )
            if ss_on_dve:
                nc.vector.tensor_tensor(scr_ss[:], mab[:], mab[:], OP.mult)
                nc.vector.tensor_scalar(
                    scr_cnt[:], scr_ss[:], 1.0, 0.0, OP.mult, OP.add,
                    accum_out=ss_ap,
                )
            else:
                nc.scalar.activation(
                    scr_act[:], mab[:], AF.Square, accum_out=ss_ap
                )

        for i, (which, c) in enumerate(cats):
            fc = float(c)
            if which == "m":
                cat_bf, cnt, s, ss = mcc_bf, cnt_m, s_m, ss_m
            else:
                cat_bf, cnt, s, ss = tr_bf, cnt_t, s_t, ss_t
            col = slice(c, c + 1)
            # cnt + mask on DVE (4x tensor_scalar)
            mk = msk[gidx % n_msk]
            nc.vector.tensor_scalar(
                mk[:], cat_bf[:], fc, 0.0, OP.is_equal, OP.add,
                accum_out=cnt[:, col],
            )
            # masked_a
            mab = ma[gidx % n_ma]
            gidx += 1
            ss_on_dve = i in ss_dve_set
            if i in ma_pool_set:
                nc.gpsimd.tensor_tensor(mab[:], mk[:], a_bf[:], OP.mult)
                lag = defer
            else:
                nc.vector.tensor_tensor(mab[:], mk[:], a_bf[:], OP.mult)
                lag = defer_dve
            if lag == 0:
                emit_consumers(mab, s[:, col], ss[:, col], ss_on_dve)
            else:
                pending.setdefault(i + lag, []).append(
                    (mab, s[:, col], ss[:, col], ss_on_dve)
                )
            for args in pending.pop(i, ()):
                emit_consumers(*args)
        for step in sorted(pending):
            for args in pending.pop(step, ()):
                emit_consumers(*args)

        # derive the last category of each group from row totals:
        #   cnt_last = T - sum(cnt), s_last = s_row - sum(s), ss_last = ss_row - sum(ss)
        dtmp = work.tile([PT, 1], F32, tag="dtmp")
        for cnt, s, ss, C in ((cnt_m, s_m, ss_m, C_MCC), (cnt_t, s_t, ss_t, C_TR)):
            last = slice(C - 1, C)
            head = slice(0, C - 1)
            nc.vector.tensor_reduce(dtmp[:], cnt[:, head], AX, OP.add)
            nc.vector.tensor_scalar(
                cnt[:, last], dtmp[:], float(T), -1.0, OP.subtract, OP.mult
            )
            nc.vector.tensor_reduce(dtmp[:], s[:, head], AX, OP.add)
            nc.vector.tensor_tensor(s[:, last], s_row[:], dtmp[:], OP.subtract)
            nc.vector.tensor_reduce(dtmp[:], ss[:, head], AX, OP.add)
            nc.vector.tensor_tensor(ss[:, last], ss_row[:], dtmp[:], OP.subtract)

        out_tile = acc_pool.tile([PT, OUT_COLS], F32, tag="out_tile")
        # col 0: seq_lens
        nc.vector.tensor_copy(out_tile[:, 0:1], seq_f[:])
        # col 1: s_row
        nc.vector.tensor_copy(out_tile[:, 1:2], s_row[:])
        # col 2: mean = s/(sl+EPS); col 3: std
        tmp1 = work.tile([PT, 1], F32, tag="tmp1")
        rec1 = work.tile([PT, 1], F32, tag="rec1")
        nc.vector.tensor_scalar(tmp1[:], seq_f[:], EPS, None, OP.add)
        nc.vector.reciprocal(rec1[:], tmp1[:])
        mean_row = out_tile[:, 2:3]
        nc.vector.tensor_tensor(mean_row, s_row[:], rec1[:], OP.mult)
        nc.vector.tensor_tensor(tmp1[:], s_row[:], mean_row, OP.mult)
        nc.vector.tensor_tensor(tmp1[:], ss_row[:], tmp1[:], OP.subtract)
        nc.vector.tensor_scalar(tmp1[:], tmp1[:], 0.0, None, OP.max)
        nc.vector.tensor_scalar(rec1[:], seq_f[:], 1.0, 0.0, OP.subtract, OP.max)
        nc.vector.tensor_scalar(rec1[:], rec1[:], EPS, None, OP.add)
        nc.vector.reciprocal(rec1[:], rec1[:])
        nc.vector.tensor_tensor(tmp1[:], tmp1[:], rec1[:], OP.mult)
        nc.scalar.sqrt(out_tile[:, 3:4], tmp1[:])

        dist_m = _cat_stats_postproc(nc, work, cnt_m, s_m, ss_m, out_tile, 4, C_MCC)
        dist_t = _cat_stats_postproc(
            nc, work, cnt_t, s_t, ss_t, out_tile, 4 + 3 * C_MCC, C_TR
        )
        nc.vector.tensor_copy(out_tile[:, 454:455], dist_m[:])
        nc.vector.tensor_copy(out_tile[:, 455:456], dist_t[:])

        nc.sync.dma_start(out_d[rows, :], out_tile[:])


_CACHED_NC = None


def _get_nc():
    global _CACHED_NC
    if _CACHED_NC is None:
        nc = bacc.Bacc(
            "TRN2",
            target_bir_lowering=False,
            debug=False,
            num_devices=NCORES,
        )
        with ExitStack() as ctx:
            tc = ctx.enter_context(tile.TileContext(nc))
            _build_body(ctx, tc)
        nc.finalize()
        _CACHED_NC = nc
    return _CACHED_NC


def kernel(amount, mcc, tr_type, seq_lens, trace=False, **trace_kwargs):
    nc = _get_nc()
    in_maps = []
    for i in range(NCORES):
        rows = slice(i * RPC, (i + 1) * RPC)
        in_maps.append(
            {
                "amount": np.ascontiguousarray(amount[rows], dtype=np.float32),
                "mcc": np.ascontiguousarray(mcc[rows], dtype=np.int32),
                "tr_type": np.ascontiguousarray(tr_type[rows], dtype=np.int32),
                "seq_lens": np.ascontiguousarray(
                    seq_lens[rows].reshape(RPC, 1), dtype=np.int32
                ),
            }
        )
    res = run_bass_kernel_spmd(
        nc, in_maps, list(range(NCORES)), trace=trace, **trace_kwargs
    )
    out = np.concatenate([r["out"] for r in res.results], axis=0)
    if trace:
        kernel.last_result = res
    return out

